# revision 10
# baseline (speedup 1.0000x reference)
"""CRPS loss kernel for Trainium2 (8 NeuronCores, SPMD data-parallel).

loss = mean(|y_pred - y|) - sum_{i,k,l} |x[i,k]-x[i,l]| / (n*2*m^2)

Key identity: for each row sorted ascending x_(0) <= ... <= x_(m-1),
    sum_{k,l} |x_k - x_l|  (all ordered pairs)  =  2 * sum_k (2k-m+1) * x_(k)
so the O(m^2) pairwise term reduces to a per-row sort (bitonic network on the
vector engine) plus a fixed weighted sum, which we fold into per-sorted-position
column sums (TensorE ones-matmul) and finish on the host in float64.

Sharding: row-parallel, 4096 rows -> 8 cores x 512 rows. Each core lays its
512 rows out as [128 partitions, 4 groups x 256] in SBUF and sorts all four
groups in parallel with batched strided access patterns (2 DVE ops per network
stage: one min, one max; merges use the all-ascending "reverse second run"
bitonic variant, the reversal folded into a negative-stride AP read).

Perf structure:
- sort runs in fp16 (2x DVE mode on most stages); the f32->fp16 conversion is
  fused into the first compare-exchange stage, which is split per group so it
  can start as soon as that group's DMAs land.
- input DMAs are interleaved across the two HWDGE-capable sequencers (SP and
  Activation) because DMA *issue* costs ~650ns each and serializes per engine.
- the final compare-exchange stage drops its min op: with linear weights,
  w_{2t}*min + w_{2t+1}*max = w_{2t}*(a+b) + 2*max, so the pre-final array's
  column sums (TensorE) plus a max-only op per group suffice.
- MAE term runs on ScalarE (|x-y| via Abs with per-partition bias, fused
  free-dim reduce) entirely inside the sort window.
- sort-order mistakes are impossible in fp16 (compare-exchange on rounded
  values is still a valid sort); value rounding adds <1e-6 relative error.
"""

import numpy as np

N, M = 4096, 256
NCORES = 8
RS = N // NCORES  # rows per core = 512
P = 128  # SBUF partitions
G = RS // P  # row groups per core = 4
W = G * M  # free-dim width = 1024
LOGM = 8  # log2(M)

_CACHE = {}


def _rawap(bass, t_ap, extra_off, free_dims):
    """AP over tile `t_ap`'s tensor with explicit free dims [[step,count],...]."""
    return bass.AP(
        t_ap.tensor, t_ap.offset + extra_off, [list(t_ap.ap[0])] + free_dims
    )


def _emit_sort(nc, bass, mybir, src_ap, bufs):
    """Bitonic network: src f32 [128,W] -> (pre-final fp16 array, max-op APs).

    Emits every stage except the final d=1 compare-exchange, for which only
    the per-group max ops are emitted (min is algebraically redundant for the
    weighted sum). Returns (prefinal_ap, [max_out_ap per group]).
    """
    MIN = mybir.AluOpType.min
    MAX = mybir.AluOpType.max
    cur = src_ap
    pp = list(bufs)

    def next_dst():
        d = pp.pop(0)
        pp.append(d)
        return d

    for k in range(1, LOGM + 1):
        K = 1 << k
        R = K >> 1
        # --- merge stage: pairs (i, K-1-i) within each K-block, i in [0,R) ---
        dst = next_dst()
        if k == 1:
            # per group: starts as soon as that group's input DMAs land, and
            # fuses the f32 -> fp16 conversion into the first min/max.
            blocks = [K, M // K]
            for g in range(G):
                off = g * M
                in_lo = _rawap(bass, cur, off, [blocks, [1, R]])
                in_hi = _rawap(bass, cur, off + K - 1, [blocks, [-1, R]])
                out_min = _rawap(bass, dst, off, [blocks, [1, R]])
                out_max = _rawap(bass, dst, off + R, [blocks, [1, R]])
                nc.vector.tensor_tensor(out_min, in_lo, in_hi, op=MIN)
                nc.vector.tensor_tensor(out_max, in_lo, in_hi, op=MAX)
        else:
            blocks = [K, W // K]
            in_lo = _rawap(bass, cur, 0, [blocks, [1, R]])
            in_hi = _rawap(bass, cur, K - 1, [blocks, [-1, R]])
            out_min = _rawap(bass, dst, 0, [blocks, [1, R]])
            out_max = _rawap(bass, dst, R, [blocks, [1, R]])
            nc.vector.tensor_tensor(out_min, in_lo, in_hi, op=MIN)
            nc.vector.tensor_tensor(out_max, in_lo, in_hi, op=MAX)
        cur = dst
        # --- halving stages: distance d = K/4 .. 1, pairs (i, i+d) ---
        for j in range(k - 2, -1, -1):
            d = 1 << j
            if k == LOGM and j == 0:
                # final stage: max-only, per group, contiguous output
                dst = next_dst()
                maxes = []
                for g in range(G):
                    off = g * M
                    in_lo = _rawap(bass, cur, off, [[2, M // 2]])
                    in_hi = _rawap(bass, cur, off + 1, [[2, M // 2]])
                    out_max = _rawap(bass, dst, off, [[1, M // 2]])
                    nc.vector.tensor_tensor(out_max, in_lo, in_hi, op=MAX)
                    maxes.append(out_max)
                return cur, maxes
            dst = next_dst()
            blocks = [2 * d, W // (2 * d)]
            in_lo = _rawap(bass, cur, 0, [blocks, [1, d]])
            in_hi = _rawap(bass, cur, d, [blocks, [1, d]])
            out_min = _rawap(bass, dst, 0, [blocks, [1, d]])
            out_max = _rawap(bass, dst, d, [blocks, [1, d]])
            nc.vector.tensor_tensor(out_min, in_lo, in_hi, op=MIN)
            nc.vector.tensor_tensor(out_max, in_lo, in_hi, op=MAX)
            cur = dst
    raise AssertionError("unreachable")


def build_nc(debug_sorted=False):
    import concourse.bass as bass
    import concourse.mybir as mybir
    import concourse.tile as tile
    from concourse import bacc

    f32 = mybir.dt.float32
    f16 = mybir.dt.float16
    nc = bacc.Bacc("TRN2", target_bir_lowering=False, debug=False)
    yp = nc.dram_tensor("yp", [RS, M], f32, kind="ExternalInput")
    yy = nc.dram_tensor("yy", [RS, 1], f32, kind="ExternalInput")
    # o_cs[0] = per-position column sums of the pre-final array (256),
    # o_cs[1,:M//2] = per-pair-slot column sums of the final max op (128).
    o_cs = nc.dram_tensor("o_cs", [2, M], f32, kind="ExternalOutput")
    o_mae = nc.dram_tensor("o_mae", [P, G], f32, kind="ExternalOutput")
    if debug_sorted:
        o_pre = nc.dram_tensor("o_pre", [P, W], f16, kind="ExternalOutput")

    with tile.TileContext(nc) as tc:
        with (
            tc.tile_pool(name="sb", bufs=1) as pool,
            tc.tile_pool(name="ps", bufs=1, space="PSUM") as pp,
        ):
            A = pool.tile([P, W], f32)
            B = pool.tile([P, W], f16)
            C = pool.tile([P, W], f16)
            Av = A[:].rearrange("p (g k) -> p g k", g=G)
            ypv = yp[:].rearrange("(g p) k -> p g k", p=P)
            # Interleave column-half chunks across SP and ACT sequencers:
            # DMA issue costs ~650ns each and serializes per engine.
            h = M // 2
            for g in range(G):
                nc.sync.dma_start(Av[:, g, :h], ypv[:, g, :h])
                nc.scalar.dma_start(Av[:, g, h:], ypv[:, g, h:])

            # y column loads: contiguous 512B each, cheap descriptors.
            Y = pool.tile([P, G], f32)
            yv = yy[:].rearrange("(g p) o -> g p o", p=P)
            for g in range(G):
                nc.sync.dma_start(Y[:, g : g + 1], yv[g])

            # MAE term on ScalarE: |x - y| with per-partition bias, fused reduce.
            negY = pool.tile([P, G], f32)
            nc.scalar.mul(negY[:], Y[:], -1.0)
            mae = pool.tile([P, G], f32)
            scratch = pool.tile([P, M], f32)
            for g in range(G):
                nc.scalar.activation(
                    scratch[:],
                    Av[:, g, :],
                    mybir.ActivationFunctionType.Abs,
                    bias=negY[:, g : g + 1],
                    scale=1.0,
                    accum_out=mae[:, g : g + 1],
                )
            nc.scalar.dma_start(o_mae[:], mae[:])

            # Bitonic sort (DVE) of all 4 groups in parallel, f32 -> fp16.
            PRE, MAXES = _emit_sort(nc, bass, mybir, A[:], [B[:], C[:]])

            if debug_sorted:
                nc.sync.dma_start(o_pre[:], PRE)

            # Column sums over partitions via accumulating ones-matmuls.
            ones = pool.tile([P, 1], f16)
            nc.gpsimd.memset(ones[:], 1.0)
            psA = pp.tile([1, M], f32)
            psB = pp.tile([1, M // 2], f32)
            for g in range(G):
                rhs = _rawap(bass, PRE, g * M, [[1, M]])
                nc.tensor.matmul(
                    psA[:], ones[:], rhs, start=(g == 0), stop=(g == G - 1)
                )
            for g in range(G):
                nc.tensor.matmul(
                    psB[:], ones[:], MAXES[g], start=(g == 0), stop=(g == G - 1)
                )
            cs_a = pool.tile([1, M], f32)
            cs_b = pool.tile([1, M // 2], f32)
            nc.scalar.copy(cs_a[:], psA[:])
            nc.scalar.copy(cs_b[:], psB[:])
            nc.sync.dma_start(o_cs[0:1, :], cs_a[:])
            nc.sync.dma_start(o_cs[1:2, : M // 2], cs_b[:])
    nc.compile()
    return nc


def _get_nc():
    if "nc" not in _CACHE:
        _CACHE["nc"] = build_nc()
    return _CACHE["nc"]


def make_in_maps(y_pred, y):
    y_pred = np.ascontiguousarray(np.asarray(y_pred, dtype=np.float32))
    y = np.ascontiguousarray(np.asarray(y, dtype=np.float32))
    assert y_pred.shape == (N, M) and y.shape == (N, 1)
    in_maps = []
    for c in range(NCORES):
        in_maps.append(
            {
                "yp": y_pred[c * RS : (c + 1) * RS],
                "yy": y[c * RS : (c + 1) * RS],
            }
        )
    return in_maps


def reduce_outputs(results):
    """Host-side final reduction in float64.

    Per row with pre-final array P (sorted up to adjacent pairs) and final
    pair maxes M_t = max(P_2t, P_2t+1):
      sum_k w_k x_(k) = sum_t [ w_2t * (P_2t + P_2t+1) + 2 * M_t ],  w_k = 2k-m+1.
    """
    w_even = (2.0 * np.arange(0, M, 2) - (M - 1)).astype(np.float64)  # w_{2t}
    mae_num = 0.0
    mix_num = 0.0
    for r in results:
        cs = r["o_cs"].astype(np.float64)
        pre = cs[0]  # per-position colsums of pre-final array, length M
        mx = cs[1, : M // 2]  # per-pair colsums of maxes, length M/2
        pairsum = pre[0::2] + pre[1::2]
        mix_num += float((w_even * pairsum).sum() + 2.0 * mx.sum())
        mae_num += float(r["o_mae"].astype(np.float64).sum())
    mae = mae_num / (N * M)
    mix = mix_num / (N * M * M)
    return np.float32(mae - mix)


def kernel(y_pred, y):
    from concourse.bass_utils import run_bass_kernel_spmd

    nc = _get_nc()
    in_maps = make_in_maps(y_pred, y)
    res = run_bass_kernel_spmd(nc, in_maps, core_ids=list(range(NCORES)))
    return reduce_outputs(res.results)


# revision 12
# speedup vs baseline: 1.0571x; 1.0571x over previous
"""CRPS loss kernel for Trainium2 (8 NeuronCores, SPMD data-parallel).

loss = mean(|y_pred - y|) - sum_{i,k,l} |x[i,k]-x[i,l]| / (n*2*m^2)

Key identity: for each row sorted ascending x_(0) <= ... <= x_(m-1),
    sum_{k,l} |x_k - x_l|  (all ordered pairs)  =  2 * sum_k (2k-m+1) * x_(k)
so the O(m^2) pairwise term reduces to a per-row sort (bitonic network on the
vector engine) plus a fixed weighted sum, which we fold into per-sorted-position
column sums (TensorE ones-matmul) and finish on the host in float64.

Sharding: row-parallel, 4096 rows -> 8 cores x 512 rows. Each core lays its
512 rows out as [128 partitions, 4 groups x 256] in SBUF and sorts all four
groups in parallel with batched strided access patterns (2 DVE ops per network
stage: one min, one max; merges use the all-ascending "reverse second run"
bitonic variant, the reversal folded into a negative-stride AP read).

Perf structure:
- sort runs in fp16 (2x DVE mode on most stages); the f32->fp16 conversion is
  fused into the first compare-exchange stage, which is split per group so it
  can start as soon as that group's DMAs land.
- input DMAs are interleaved across the two HWDGE-capable sequencers (SP and
  Activation) because DMA *issue* costs ~650ns each and serializes per engine.
- the final compare-exchange stage drops its min op: with linear weights,
  w_{2t}*min + w_{2t+1}*max = w_{2t}*(a+b) + 2*max, so the pre-final array's
  column sums (TensorE) plus a max-only op per group suffice.
- MAE term runs on ScalarE (|x-y| via Abs with per-partition bias, fused
  free-dim reduce) entirely inside the sort window.
- sort-order mistakes are impossible in fp16 (compare-exchange on rounded
  values is still a valid sort); value rounding adds <1e-6 relative error.
"""

import numpy as np

N, M = 4096, 256
NCORES = 8
RS = N // NCORES  # rows per core = 512
P = 128  # SBUF partitions
G = RS // P  # row groups per core = 4
W = G * M  # free-dim width = 1024
LOGM = 8  # log2(M)

_CACHE = {}


def _rawap(bass, t_ap, extra_off, free_dims):
    """AP over tile `t_ap`'s tensor with explicit free dims [[step,count],...]."""
    return bass.AP(
        t_ap.tensor, t_ap.offset + extra_off, [list(t_ap.ap[0])] + free_dims
    )


def _emit_sort(nc, bass, mybir, src_ap, bufs):
    """Bitonic network: src f32 [128,W] -> (pre-final fp16 array, max-op APs).

    Emits every stage except the final d=1 compare-exchange, for which only
    the per-group max ops are emitted (min is algebraically redundant for the
    weighted sum). Returns (prefinal_ap, [max_out_ap per group]).
    """
    MIN = mybir.AluOpType.min
    MAX = mybir.AluOpType.max
    cur = src_ap
    pp = list(bufs)

    def next_dst():
        d = pp.pop(0)
        pp.append(d)
        return d

    for k in range(1, LOGM + 1):
        K = 1 << k
        R = K >> 1
        # --- merge stage: pairs (i, K-1-i) within each K-block, i in [0,R) ---
        dst = next_dst()
        if k == 1:
            # per group: starts as soon as that group's input DMAs land, and
            # fuses the f32 -> fp16 conversion into the first min/max.
            blocks = [K, M // K]
            for g in range(G):
                off = g * M
                in_lo = _rawap(bass, cur, off, [blocks, [1, R]])
                in_hi = _rawap(bass, cur, off + K - 1, [blocks, [-1, R]])
                out_min = _rawap(bass, dst, off, [blocks, [1, R]])
                out_max = _rawap(bass, dst, off + R, [blocks, [1, R]])
                nc.vector.tensor_tensor(out_min, in_lo, in_hi, op=MIN)
                nc.vector.tensor_tensor(out_max, in_lo, in_hi, op=MAX)
        else:
            blocks = [K, W // K]
            in_lo = _rawap(bass, cur, 0, [blocks, [1, R]])
            in_hi = _rawap(bass, cur, K - 1, [blocks, [-1, R]])
            out_min = _rawap(bass, dst, 0, [blocks, [1, R]])
            out_max = _rawap(bass, dst, R, [blocks, [1, R]])
            nc.vector.tensor_tensor(out_min, in_lo, in_hi, op=MIN)
            nc.vector.tensor_tensor(out_max, in_lo, in_hi, op=MAX)
        cur = dst
        # --- halving stages: distance d = K/4 .. 1, pairs (i, i+d) ---
        for j in range(k - 2, -1, -1):
            d = 1 << j
            if k == LOGM and j == 0:
                # final stage: max-only, per group, contiguous output
                dst = next_dst()
                maxes = []
                for g in range(G):
                    off = g * M
                    in_lo = _rawap(bass, cur, off, [[2, M // 2]])
                    in_hi = _rawap(bass, cur, off + 1, [[2, M // 2]])
                    out_max = _rawap(bass, dst, off, [[1, M // 2]])
                    nc.vector.tensor_tensor(out_max, in_lo, in_hi, op=MAX)
                    maxes.append(out_max)
                return cur, maxes
            dst = next_dst()
            blocks = [2 * d, W // (2 * d)]
            in_lo = _rawap(bass, cur, 0, [blocks, [1, d]])
            in_hi = _rawap(bass, cur, d, [blocks, [1, d]])
            out_min = _rawap(bass, dst, 0, [blocks, [1, d]])
            out_max = _rawap(bass, dst, d, [blocks, [1, d]])
            nc.vector.tensor_tensor(out_min, in_lo, in_hi, op=MIN)
            nc.vector.tensor_tensor(out_max, in_lo, in_hi, op=MAX)
            cur = dst
    raise AssertionError("unreachable")


def build_nc(debug_sorted=False):
    import concourse.bass as bass
    import concourse.mybir as mybir
    import concourse.tile as tile
    from concourse import bacc

    f32 = mybir.dt.float32
    f16 = mybir.dt.float16
    nc = bacc.Bacc("TRN2", target_bir_lowering=False, debug=False)
    yp = nc.dram_tensor("yp", [RS, M], f32, kind="ExternalInput")
    yy = nc.dram_tensor("yy", [RS, 1], f32, kind="ExternalInput")
    # o_cs[0, :M] = per-position column sums of the pre-final array (256),
    # o_cs[0, M:] = per-pair-slot column sums of the final max op (128).
    o_cs = nc.dram_tensor("o_cs", [1, M + M // 2], f32, kind="ExternalOutput")
    o_mae = nc.dram_tensor("o_mae", [P, G], f32, kind="ExternalOutput")
    if debug_sorted:
        o_pre = nc.dram_tensor("o_pre", [P, W], f16, kind="ExternalOutput")

    with tile.TileContext(nc) as tc:
        with (
            tc.tile_pool(name="sb", bufs=1) as pool,
            tc.tile_pool(name="ps", bufs=1, space="PSUM") as pp,
        ):
            A = pool.tile([P, W], f32)
            B = pool.tile([P, W], f16)
            C = pool.tile([P, W], f16)
            Av = A[:].rearrange("p (g k) -> p g k", g=G)
            ypv = yp[:].rearrange("(g p) k -> p g k", p=P)
            # One DMA per group, alternating SP/ACT sequencers: DMA *issue*
            # costs ~0.7-1.3us each and serializes per engine, so fewer,
            # larger transfers get the last group into SBUF soonest.
            for g in range(G):
                eng = nc.sync if g % 2 == 0 else nc.scalar
                eng.dma_start(Av[:, g, :], ypv[:, g, :])

            # y column loads: contiguous 512B each, cheap descriptors.
            Y = pool.tile([P, G], f32)
            yv = yy[:].rearrange("(g p) o -> g p o", p=P)
            for g in range(G):
                nc.sync.dma_start(Y[:, g : g + 1], yv[g])

            # MAE term on ScalarE: |x - y| with per-partition bias, fused reduce.
            negY = pool.tile([P, G], f32)
            nc.scalar.mul(negY[:], Y[:], -1.0)
            mae = pool.tile([P, G], f32)
            scratch = pool.tile([P, M], f32)
            for g in range(G):
                nc.scalar.activation(
                    scratch[:],
                    Av[:, g, :],
                    mybir.ActivationFunctionType.Abs,
                    bias=negY[:, g : g + 1],
                    scale=1.0,
                    accum_out=mae[:, g : g + 1],
                )
            nc.scalar.dma_start(o_mae[:], mae[:])

            # Bitonic sort (DVE) of all 4 groups in parallel, f32 -> fp16.
            PRE, MAXES = _emit_sort(nc, bass, mybir, A[:], [B[:], C[:]])

            if debug_sorted:
                nc.sync.dma_start(o_pre[:], PRE)

            # Column sums over partitions via accumulating ones-matmuls.
            ones = pool.tile([P, 1], f16)
            nc.gpsimd.memset(ones[:], 1.0)
            psA = pp.tile([1, M], f32)
            psB = pp.tile([1, M // 2], f32)
            for g in range(G):
                rhs = _rawap(bass, PRE, g * M, [[1, M]])
                nc.tensor.matmul(
                    psA[:], ones[:], rhs, start=(g == 0), stop=(g == G - 1)
                )
            for g in range(G):
                nc.tensor.matmul(
                    psB[:], ones[:], MAXES[g], start=(g == 0), stop=(g == G - 1)
                )
            cs = pool.tile([1, M + M // 2], f32)
            nc.scalar.copy(cs[:, :M], psA[:])
            nc.vector.tensor_copy(cs[:, M:], psB[:])
            nc.sync.dma_start(o_cs[:], cs[:])
    nc.compile()
    return nc


def _get_nc():
    if "nc" not in _CACHE:
        _CACHE["nc"] = build_nc()
    return _CACHE["nc"]


def make_in_maps(y_pred, y):
    y_pred = np.ascontiguousarray(np.asarray(y_pred, dtype=np.float32))
    y = np.ascontiguousarray(np.asarray(y, dtype=np.float32))
    assert y_pred.shape == (N, M) and y.shape == (N, 1)
    in_maps = []
    for c in range(NCORES):
        in_maps.append(
            {
                "yp": y_pred[c * RS : (c + 1) * RS],
                "yy": y[c * RS : (c + 1) * RS],
            }
        )
    return in_maps


def reduce_outputs(results):
    """Host-side final reduction in float64.

    Per row with pre-final array P (sorted up to adjacent pairs) and final
    pair maxes M_t = max(P_2t, P_2t+1):
      sum_k w_k x_(k) = sum_t [ w_2t * (P_2t + P_2t+1) + 2 * M_t ],  w_k = 2k-m+1.
    """
    w_even = (2.0 * np.arange(0, M, 2) - (M - 1)).astype(np.float64)  # w_{2t}
    mae_num = 0.0
    mix_num = 0.0
    for r in results:
        cs = r["o_cs"].astype(np.float64).reshape(-1)
        pre = cs[:M]  # per-position colsums of pre-final array, length M
        mx = cs[M:]  # per-pair colsums of maxes, length M/2
        pairsum = pre[0::2] + pre[1::2]
        mix_num += float((w_even * pairsum).sum() + 2.0 * mx.sum())
        mae_num += float(r["o_mae"].astype(np.float64).sum())
    mae = mae_num / (N * M)
    mix = mix_num / (N * M * M)
    return np.float32(mae - mix)


def kernel(y_pred, y):
    from concourse.bass_utils import run_bass_kernel_spmd

    nc = _get_nc()
    in_maps = make_in_maps(y_pred, y)
    res = run_bass_kernel_spmd(nc, in_maps, core_ids=list(range(NCORES)))
    return reduce_outputs(res.results)
